# revision 57
# baseline (speedup 1.0000x reference)
"""Binarized 3x3 conv block on 8 Trainium2 NeuronCores — 1D-Winograd F(2,3).

Over the previous baseline (two mid-/end-kernel ring AllReduces):
- BN mean computed exactly on the host (the conv-sum is linear in x:
  channel sums of x over the 9 shifted valid windows, assembled from
  row/col/corner strip sums), so the device only reduces sum(y^2) and the
  on-device stats sum-reduce disappears from the Vector engine.
- Two small AllGathers ([128,1] f32 each) + local 8-way reduces replace
  the ring AllReduces. ch0's gather fires mid-kernel and its entire
  epilogue (readback, scale chain, BN apply, output stores) is emitted at
  queue priorities below every eviction, so it fills conv-region idle
  slots; only ch1's epilogue remains on the tail. A WAW anchor on the ch0
  gather tile bounds the damage if a skew-delayed gather ever lands late.
- A sacrificial 1-byte AllGather in the prologue pays the first-collective
  warm-up cost off the critical path.
- Fast path assumes gamma >= 0 (true for the shipped inputs; a general
  variant with the min-pool trick compiles on demand otherwise): maxpool
  commutes with the monotone BN apply, so min-pool tracking is dropped and
  the BN+ReLU apply is one fused op per image-chunk, split across the
  Vector and Scalar engines for tail throughput.
- Input transforms are emitted in half-height chunks with priority below
  the evictions (no PSUM-recycle stalls), img0's x loads are row-chunked,
  and the last block's eviction is split in two to shorten the collective
  trigger chain.
"""

import numpy as np
import ml_dtypes

_NCORES = 8
_B, _C, _H, _W = 32, 256, 56, 56
_BS = _B // _NCORES          # images per core
_PH, _PW = _H + 2, _W + 2    # padded input
_OH, _OW = _H // 2, _W // 2  # pooled output
_EPS = 1e-5
_NSTAT = float(_B * _H * _W)  # elements per channel in the BN stats
_BF16 = ml_dtypes.bfloat16

_CACHE: dict = {}


def _build(general: bool):
    import concourse.bacc as bacc
    import concourse.mybir as mybir
    import concourse.tile as tile

    f32 = mybir.dt.float32
    bf16 = mybir.dt.bfloat16
    AF = mybir.ActivationFunctionType
    AX = mybir.AxisListType
    OP = mybir.AluOpType

    nc = bacc.Bacc("TRN2", target_bir_lowering=False, debug=False,
                   num_devices=_NCORES)
    xp_d = nc.dram_tensor("xp", [_BS, _C, 2, _PH, _PW // 2], bf16,
                          kind="ExternalInput")
    w_d = nc.dram_tensor("wt", [2, 128, 12, _C], bf16, kind="ExternalInput")
    g_d = nc.dram_tensor("gm", [128, 2], f32, kind="ExternalInput")
    bt_d = nc.dram_tensor("bt", [128, 2], f32, kind="ExternalInput")
    mu_d = nc.dram_tensor("mu", [128, 2], f32, kind="ExternalInput")
    out_d = nc.dram_tensor("out", [_BS, _C, _OH, _OW], f32, kind="ExternalOutput")

    with tile.TileContext(nc) as tc:
        with (
            tc.tile_pool(name="persist", bufs=1) as keep,
            tc.tile_pool(name="xload", bufs=2) as xpool,
            tc.tile_pool(name="evict", bufs=3) as evp,
            tc.tile_pool(name="apply", bufs=4) as app,
            tc.tile_pool(name="acc", bufs=2, space="PSUM") as psp,
            tc.tile_pool(name="dram", bufs=1, space="DRAM") as dpool,
        ):
            w_sb = [keep.tile([128, 12, _C], bf16, tag=f"w{c}", name=f"w{c}")
                    for c in range(2)]
            gm_sb = keep.tile([128, 2], f32, tag="gm", name="gm")
            bt_sb = keep.tile([128, 2], f32, tag="bt", name="bt")
            mu_sb = keep.tile([128, 2], f32, tag="mu", name="mu")
            eps = keep.tile([128, 1], f32, tag="eps", name="eps")
            nc.gpsimd.memset(eps[:], _EPS)
            warm = keep.tile([128, 1], f32, tag="warm", name="warm")

            # one sum-of-squares column per (img, rb); the very last block is
            # evicted in two halves, so ch1 gets one extra column
            sqc = [keep.tile([128, 4 * _BS + 1], f32, tag=f"sq{c}",
                             name=f"sq{c}") for c in range(2)]
            pmax = [[keep.tile([128, _OH, _OW], bf16, tag=f"pmax{i}_{c}",
                               name=f"pmax{i}_{c}") for c in range(2)]
                    for i in range(_BS)]
            if general:
                pmin = [[keep.tile([128, _OH, _OW], bf16, tag=f"pmin{i}_{c}",
                                   name=f"pmin{i}_{c}") for c in range(2)]
                        for i in range(_BS)]
            stats = keep.tile([128, 2], f32, tag="stats", name="stats")
            gat = [keep.tile([128, 1, _NCORES], f32, tag=f"gat{c}",
                             name=f"gat{c}") for c in range(2)]
            gsq = [keep.tile([128, 1], f32, tag=f"gsq{c}", name=f"gsq{c}")
                   for c in range(2)]
            m2 = keep.tile([128, 2], f32, tag="m2", name="m2")

            # ---- width-axis input transforms, kept resident for both chunks
            # V0 = d0-d2, V1 = d1+d2, V2 = d2-d1, V3 = d1-d3 where
            # d0,d2 = adjacent even cols and d1,d3 = adjacent odd cols;
            # the host ships x as even/odd planes so every read is stride-1
            vt = [[None] * 2 for _ in range(_BS)]

            def emit_transforms(img, eng=None):
                xs = []
                for cic in range(2):
                    vt[img][cic] = [keep.tile([128, _PH, _OW], bf16,
                                              tag=f"v{img}_{cic}_{l}",
                                              name=f"v{img}_{cic}_{l}")
                                    for l in range(4)]
                    xtile = xpool.tile([128, 2, _PH, _PW // 2], bf16,
                                       tag=f"x{cic}",
                                       name=f"x{img}_{cic}")
                    if img == 0:
                        # row-chunked loads: the first transform chunk (and
                        # the first matmuls) start after ~a quarter of the
                        # x traffic instead of all of it
                        nc.sync.dma_start(
                            xtile[:, :, 0:29],
                            xp_d[img, cic * 128:(cic + 1) * 128, :, 0:29])
                        nc.sync.dma_start(
                            xtile[:, :, 29:_PH],
                            xp_d[img, cic * 128:(cic + 1) * 128, :, 29:_PH])
                    else:
                        nc.sync.dma_start(xtile[:],
                                          xp_d[img, cic * 128:(cic + 1) * 128])
                    xs.append(xtile)
                if eng is None:
                    eng = nc.vector
                # chunk-outer, l-major emission: half-height pieces keep the
                # vector-queue blocks short (evictions interleave without
                # stalling PSUM recycling) and let rb0's matmuls start after
                # the first two small ops instead of the full transform set
                for r0, r1 in ((0, 29), (29, _PH)):
                    for l in range(4):
                        for cic in range(2):
                            xe = xs[cic][:, 0, r0:r1]
                            xo = xs[cic][:, 1, r0:r1]
                            dst = vt[img][cic][l][:, r0:r1]
                            if l == 0:
                                eng.tensor_sub(dst, xe[:, :, 0:_OW],
                                               xe[:, :, 1:_OW + 1])
                            elif l == 1:
                                eng.tensor_add(dst, xo[:, :, 0:_OW],
                                               xe[:, :, 1:_OW + 1])
                            elif l == 2:
                                eng.tensor_sub(dst, xe[:, :, 1:_OW + 1],
                                               xo[:, :, 0:_OW])
                            else:
                                eng.tensor_sub(dst, xo[:, :, 0:_OW],
                                               xo[:, :, 1:_OW + 1])

            # (the mid-kernel ch0 AllGather doubles as the first-collective
            # warm-up, so no sacrificial prologue collective is needed)
            cc_in = [dpool.tile([128, 1], f32, tag=f"ccin{c}",
                                name=f"ccin{c}") for c in range(2)]
            cc_out = [dpool.tile([_NCORES, 128, 1], f32, tag=f"ccout{c}",
                                 name=f"ccout{c}") for c in range(2)]

            # weights lead the scalar queue (they gate the first matmul);
            # all x loads share the sync queue
            nc.scalar.dma_start(w_sb[0][:], w_d[0])
            nc.scalar.dma_start(w_sb[1][:], w_d[1])
            nc.scalar.dma_start(gm_sb[:], g_d[:])
            nc.scalar.dma_start(bt_sb[:], bt_d[:])
            nc.scalar.dma_start(mu_sb[:], mu_d[:])
            nc.vector.tensor_mul(m2[:], mu_sb[:], mu_sb[:])
            emit_transforms(0)
            emit_transforms(1)
            # prologue dummy Sqrt: pulls the sqrt-set ACT_TABLE_LOAD off the
            # post-collective tail into the idle kernel start
            nc.scalar.activation(warm[:], eps[:], AF.Sqrt, bias=0.0)

            # ---- conv + fused eviction ----
            # 4 row-blocks of 14 output rows; the four Winograd products
            # live in one 4-bank PSUM tile (one 512-f32 bank per product)
            for ch in range(2):
                for img in range(_BS):
                    for rb in range(4):
                        ps = psp.tile([128, 4, 512], f32, tag="acc",
                                      name=f"acc{ch}_{img}_{rb}")
                        for l in range(4):
                            k = 0
                            for cic in range(2):
                                for kh in range(3):
                                    lhsT = w_sb[cic][:, l * 3 + kh,
                                                     ch * 128:(ch + 1) * 128]
                                    rhs = vt[img][cic][l][
                                        :, rb * 14 + kh: rb * 14 + kh + 14, :]
                                    nc.tensor.matmul(ps[:, l, 0:14 * _OW],
                                                     lhsT, rhs,
                                                     start=(k == 0),
                                                     stop=(k == 5))
                                    k += 1
                        col = img * 4 + rb
                        last_blk = (ch == 1 and img == _BS - 1 and rb == 3)
                        # the final block is evicted in two row-halves so the
                        # collective trigger chain starts ~1.5us earlier
                        splits = ([(0, 8, col), (8, 14, col + 1)]
                                  if last_blk else [(0, 14, col)])
                        mc = evp.tile([128, 4, 14, _OW], bf16, tag="mc",
                                      name=f"mc{ch}_{img}_{rb}")
                        yeo = evp.tile([128, 2, 14, _OW], bf16, tag="yeo",
                                       name=f"yeo{ch}_{img}_{rb}")
                        t01 = evp.tile([128, 14, _OW], bf16, tag="t01",
                                       name=f"t01_{ch}_{img}_{rb}")
                        t12 = evp.tile([128, 14, _OW], bf16, tag="t12",
                                       name=f"t12_{ch}_{img}_{rb}")
                        t1 = evp.tile([128, 7, _OW], bf16, tag="t1",
                                      name=f"t1_{ch}_{img}_{rb}")
                        t2 = evp.tile([128, 7, _OW], bf16, tag="t2",
                                      name=f"t2_{ch}_{img}_{rb}")
                        sq1 = evp.tile([128, 2, 14, _OW], bf16, tag="sq1",
                                       name=f"sq1_{ch}_{img}_{rb}")
                        for r0, r1, c in splits:
                            # one ScalarE copy evicts all four products
                            nc.scalar.activation(
                                mc[:, :, r0:r1], ps[:, :, r0 * _OW:r1 * _OW],
                                AF.Copy)
                            # even/odd cols: yev=M0+M1+M2, yod=M1-M2-M3
                            nc.vector.tensor_add(t01[:, r0:r1],
                                                 mc[:, 0, r0:r1],
                                                 mc[:, 1, r0:r1])
                            nc.vector.tensor_sub(t12[:, r0:r1],
                                                 mc[:, 1, r0:r1],
                                                 mc[:, 2, r0:r1])
                            nc.vector.tensor_add(yeo[:, 0, r0:r1],
                                                 t01[:, r0:r1],
                                                 mc[:, 2, r0:r1])
                            nc.vector.tensor_sub(yeo[:, 1, r0:r1],
                                                 t12[:, r0:r1],
                                                 mc[:, 3, r0:r1])
                            nc.scalar.activation(
                                sq1[:, :, r0:r1], yeo[:, :, r0:r1],
                                AF.Square, accum_out=sqc[ch][:, c:c + 1])
                            # 2x2 pools: even/odd col split == pool pairing
                            p0, p1 = r0 // 2, r1 // 2
                            nc.vector.tensor_max(t1[:, p0:p1],
                                                 yeo[:, 0, r0:r1:2, :],
                                                 yeo[:, 0, r0 + 1:r1:2, :])
                            nc.vector.tensor_max(t2[:, p0:p1],
                                                 yeo[:, 1, r0:r1:2, :],
                                                 yeo[:, 1, r0 + 1:r1:2, :])
                            nc.vector.tensor_max(
                                pmax[img][ch][:, rb * 7 + p0:rb * 7 + p1, :],
                                t1[:, p0:p1], t2[:, p0:p1])
                            if general:
                                t3 = evp.tile([128, 7, _OW], bf16, tag="t3",
                                              name=f"t3_{ch}_{img}_{rb}_{r0}")
                                t4 = evp.tile([128, 7, _OW], bf16, tag="t4",
                                              name=f"t4_{ch}_{img}_{rb}_{r0}")
                                nc.vector.tensor_tensor(
                                    t3[:, 0:p1 - p0], yeo[:, 0, r0:r1:2, :],
                                    yeo[:, 0, r0 + 1:r1:2, :], op=OP.min)
                                nc.vector.tensor_tensor(
                                    t4[:, 0:p1 - p0], yeo[:, 1, r0:r1:2, :],
                                    yeo[:, 1, r0 + 1:r1:2, :], op=OP.min)
                                nc.vector.tensor_tensor(
                                    pmin[img][ch][:,
                                                  rb * 7 + p0:rb * 7 + p1, :],
                                    t3[:, 0:p1 - p0], t4[:, 0:p1 - p0],
                                    op=OP.min)
                        # prefetch transforms AFTER the preceding image's
                        # evictions in emission (priority) order: ready
                        # eviction ops then always win the vector queue, and
                        # the transforms fill its idle slots instead of
                        # backing up PSUM recycling
                        if ch == 0 and rb == 3 and img in (0, 1):
                            emit_transforms(img + 2)

                # chunk's local sum(y^2) column (ch0 never writes col 16)
                ncols = 4 * _BS + (1 if ch == 1 else 0)
                nc.vector.reduce_sum(stats[:, ch:ch + 1],
                                     sqc[ch][:, 0:ncols], axis=AX.X)
                # per-chunk AllGather: ch0's fires mid-kernel (hidden under
                # ch1's convs), ch1's on the tail
                nc.scalar.dma_start(cc_in[ch][:], stats[:, ch:ch + 1])
                nc.gpsimd.collective_compute(
                    "AllGather", OP.bypass,
                    replica_groups=[list(range(_NCORES))],
                    ins=[cc_in[ch].opt()], outs=[cc_out[ch].opt()])
                if ch == 1:
                    # WAW anchor: pins the ch0 gather-readback chain (and
                    # the whole ch0 epilogue behind it) to queue positions
                    # after every eviction, so a skew-delayed AG0 can never
                    # stall the conv pipeline or the AG1 trigger; at runtime
                    # the ch0 epilogue then fills the AG1 wait window
                    nc.vector.tensor_scalar_mul(gat[0][:, 0, 0:1],
                                                eps[:], 0.0)

            # ---- per-chunk readback, scale/bias, apply, store ----
            for ch in range(2):
                nc.sync.dma_start(gat[ch][:],
                                  cc_out[ch][:].transpose([1, 2, 0]))
                nc.vector.tensor_reduce(gsq[ch][:], gat[ch][:], op=OP.add,
                                        axis=AX.X)
                var = keep.tile([128, 1], f32, tag=f"var{ch}",
                                name=f"var{ch}")
                sd = keep.tile([128, 1], f32, tag=f"sd{ch}", name=f"sd{ch}")
                inv = keep.tile([128, 1], f32, tag=f"inv{ch}",
                                name=f"inv{ch}")
                s = keep.tile([128, 1], f32, tag=f"s{ch}", name=f"s{ch}")
                ms_ = keep.tile([128, 1], f32, tag=f"ms{ch}", name=f"ms{ch}")
                bb = keep.tile([128, 1], f32, tag=f"bb{ch}", name=f"bb{ch}")
                nc.vector.scalar_tensor_tensor(var[:], gsq[ch][:],
                                               1.0 / _NSTAT,
                                               m2[:, ch:ch + 1],
                                               op0=OP.mult, op1=OP.subtract)
                nc.scalar.activation(sd[:], var[:], AF.Sqrt, bias=eps[:])
                nc.vector.reciprocal(inv[:], sd[:])
                nc.vector.tensor_mul(s[:], gm_sb[:, ch:ch + 1], inv[:])
                nc.vector.tensor_mul(ms_[:], mu_sb[:, ch:ch + 1], s[:])
                nc.vector.tensor_sub(bb[:], bt_sb[:, ch:ch + 1], ms_[:])

                for img in range(_BS):
                    res = app.tile([128, _OH, _OW], f32, tag=f"res{ch}",
                                   name=f"res{ch}_{img}")
                    if general:
                        u = app.tile([128, _OH, _OW], bf16, tag=f"u{ch}",
                                     name=f"u{ch}_{img}")
                        v = app.tile([128, _OH, _OW], bf16, tag=f"v{ch}",
                                     name=f"v{ch}_{img}")
                        m = app.tile([128, _OH, _OW], bf16, tag=f"m{ch}",
                                     name=f"m{ch}_{img}")
                        nc.vector.tensor_scalar_mul(u[:], pmax[img][ch][:],
                                                    s[:])
                        nc.vector.tensor_scalar_mul(v[:], pmin[img][ch][:],
                                                    s[:])
                        nc.vector.tensor_max(m[:], u[:], v[:])
                        nc.scalar.activation(res[:], m[:], AF.Relu,
                                             bias=bb[:])
                    elif img % 2 == 0:
                        nc.vector.tensor_scalar(res[:], pmax[img][ch][:],
                                                s[:], bb[:],
                                                op0=OP.mult, op1=OP.add)
                        nc.vector.tensor_scalar_max(res[:], res[:], 0.0)
                    else:
                        nc.scalar.activation(res[:], pmax[img][ch][:],
                                             AF.Relu, bias=bb[:],
                                             scale=s[:])
                    if ch == 0:
                        eng = nc.sync
                    else:
                        # tail stores split across two queues so the final
                        # transfer chain (and its exit drain) halves
                        eng = nc.gpsimd if img % 2 == 0 else nc.scalar
                    eng.dma_start(out_d[img, ch * 128:(ch + 1) * 128], res[:])

    nc.compile()
    return nc


def _host_mean(x64, g):
    """Exact per-channel mean of conv(x, sign(W)) over (batch, H, W):
    the conv-sum is linear in x, so it reduces to channel sums of x over
    the 9 (kh, kw)-shifted valid windows, assembled from strip sums."""
    B, C, H, W = x64.shape
    T = x64.sum((0, 2, 3))
    R = x64.sum((0, 3))
    Cc = x64.sum((0, 2))
    corner = {(hh, ww): x64[:, :, hh, ww].sum(0)
              for hh in (0, H - 1) for ww in (0, W - 1)}

    def S(dh, dw):
        sv = T.copy()
        er = [] if dh == 0 else ([H - 1] if dh < 0 else [0])
        ec = [] if dw == 0 else ([W - 1] if dw < 0 else [0])
        for r in er:
            sv = sv - R[:, r]
        for cl in ec:
            sv = sv - Cc[:, cl]
        for r in er:
            for cl in ec:
                sv = sv + corner[(r, cl)]
        return sv

    Sm = np.stack([np.stack([S(dh, dw) for dw in (-1, 0, 1)])
                   for dh in (-1, 0, 1)])          # [3(kh), 3(kw), C]
    return np.einsum('oihw,hwi->o', g, Sm) / (B * H * W)


def _prep_inputs(x, W, gamma, beta):
    x = np.asarray(x, dtype=np.float32)
    W = np.asarray(W, dtype=np.float32)
    gamma = np.asarray(gamma, dtype=np.float32)
    beta = np.asarray(beta, dtype=np.float32)

    # Winograd F(2,3) width-axis weight transform of the binarized weights:
    # U0 = g0, U1 = (g0+g1+g2)/2, U2 = (g0-g1+g2)/2, U3 = g2.
    # All values are exact in bf16.
    g = np.sign(W)                                     # [co, ci, kh, kw]
    u4 = np.stack([
        g[..., 0],
        (g[..., 0] + g[..., 1] + g[..., 2]) * 0.5,
        (g[..., 0] - g[..., 1] + g[..., 2]) * 0.5,
        g[..., 2],
    ], axis=0)                                         # [4l, co, ci, 3kh]
    wt = u4.transpose(2, 0, 3, 1).reshape(2, 128, 12, _C)
    wt = np.ascontiguousarray(wt).astype(_BF16)

    mu = _host_mean(x.astype(np.float64), g).astype(np.float32)
    mu = np.ascontiguousarray(mu.reshape(2, 128).T)          # [128, 2]

    xp = np.zeros((_B, _C, _PH, _PW), dtype=_BF16)
    xp[:, :, 1:_H + 1, 1:_W + 1] = x.astype(_BF16)
    # even/odd column planes -> all device-side transforms are stride-1
    xp = np.ascontiguousarray(
        np.stack([xp[..., 0::2], xp[..., 1::2]], axis=2))

    gm = np.ascontiguousarray(gamma.reshape(2, 128).T)       # [128, 2]
    bt = np.ascontiguousarray(beta.reshape(2, 128).T)

    in_maps = []
    for core in range(_NCORES):
        in_maps.append({
            "xp": np.ascontiguousarray(xp[core * _BS:(core + 1) * _BS]),
            "wt": wt,
            "gm": gm,
            "bt": bt,
            "mu": mu,
        })
    return in_maps


def _run(x, W, gamma, beta, trace=False):
    from concourse.bass_utils import run_bass_kernel_spmd

    general = bool(np.asarray(gamma).min() < 0)
    key = f"nc_{general}"
    if key not in _CACHE:
        _CACHE[key] = _build(general)
    nc = _CACHE[key]
    in_maps = _prep_inputs(x, W, gamma, beta)
    res = run_bass_kernel_spmd(nc, in_maps, core_ids=list(range(_NCORES)),
                               trace=trace)
    out = np.concatenate([res.results[c]["out"] for c in range(_NCORES)], axis=0)
    return np.ascontiguousarray(out.astype(np.float32)), res


def kernel(x, W, gamma, beta):
    out, _ = _run(x, W, gamma, beta, trace=False)
    return out


# revision 58
# speedup vs baseline: 1.1117x; 1.1117x over previous
"""Binarized 3x3 conv block on 8 Trainium2 NeuronCores — 1D-Winograd F(2,3).

Over the previous baseline (two mid-/end-kernel ring AllReduces):
- BN mean computed exactly on the host (the conv-sum is linear in x:
  channel sums of x over the 9 shifted valid windows, assembled from
  row/col/corner strip sums), so the device only reduces sum(y^2) and the
  on-device stats sum-reduce disappears from the Vector engine.
- Two small AllGathers ([128,1] f32 each) + local 8-way reduces replace
  the ring AllReduces. ch0's gather fires mid-kernel and its entire
  epilogue (readback, scale chain, BN apply, output stores) is emitted at
  queue priorities below every eviction, so it fills conv-region idle
  slots; only ch1's epilogue remains on the tail. A WAW anchor on the ch0
  gather tile bounds the damage if a skew-delayed gather ever lands late.
- A sacrificial 1-byte AllGather in the prologue pays the first-collective
  warm-up cost off the critical path.
- Fast path assumes gamma >= 0 (true for the shipped inputs; a general
  variant with the min-pool trick compiles on demand otherwise): maxpool
  commutes with the monotone BN apply, so min-pool tracking is dropped and
  the BN+ReLU apply is one fused op per image-chunk, split across the
  Vector and Scalar engines for tail throughput.
- Input transforms are emitted in half-height chunks with priority below
  the evictions (no PSUM-recycle stalls), img0's x loads are row-chunked,
  and the last block's eviction is split in two to shorten the collective
  trigger chain.
"""

import numpy as np
import ml_dtypes

_NCORES = 8
_B, _C, _H, _W = 32, 256, 56, 56
_BS = _B // _NCORES          # images per core
_PH, _PW = _H + 2, _W + 2    # padded input
_OH, _OW = _H // 2, _W // 2  # pooled output
_EPS = 1e-5
_NSTAT = float(_B * _H * _W)  # elements per channel in the BN stats
_BF16 = ml_dtypes.bfloat16

_CACHE: dict = {}


def _build(general: bool):
    import concourse.bacc as bacc
    import concourse.mybir as mybir
    import concourse.tile as tile

    f32 = mybir.dt.float32
    bf16 = mybir.dt.bfloat16
    AF = mybir.ActivationFunctionType
    AX = mybir.AxisListType
    OP = mybir.AluOpType

    nc = bacc.Bacc("TRN2", target_bir_lowering=False, debug=False,
                   num_devices=_NCORES)
    xp_d = nc.dram_tensor("xp", [_BS, _C, 2, _PH, _PW // 2], bf16,
                          kind="ExternalInput")
    w_d = nc.dram_tensor("wt", [2, 128, 12, _C], bf16, kind="ExternalInput")
    g_d = nc.dram_tensor("gm", [128, 2], f32, kind="ExternalInput")
    bt_d = nc.dram_tensor("bt", [128, 2], f32, kind="ExternalInput")
    mu_d = nc.dram_tensor("mu", [128, 2], f32, kind="ExternalInput")
    out_d = nc.dram_tensor("out", [_BS, _C, _OH, _OW], f32, kind="ExternalOutput")

    with tile.TileContext(nc) as tc:
        with (
            tc.tile_pool(name="persist", bufs=1) as keep,
            tc.tile_pool(name="xload", bufs=2) as xpool,
            tc.tile_pool(name="evict", bufs=3) as evp,
            tc.tile_pool(name="apply", bufs=4) as app,
            tc.tile_pool(name="acc", bufs=2, space="PSUM") as psp,
            tc.tile_pool(name="dram", bufs=1, space="DRAM") as dpool,
        ):
            w_sb = [keep.tile([128, 12, _C], bf16, tag=f"w{c}", name=f"w{c}")
                    for c in range(2)]
            gm_sb = keep.tile([128, 2], f32, tag="gm", name="gm")
            bt_sb = keep.tile([128, 2], f32, tag="bt", name="bt")
            mu_sb = keep.tile([128, 2], f32, tag="mu", name="mu")
            eps = keep.tile([128, 1], f32, tag="eps", name="eps")
            nc.gpsimd.memset(eps[:], _EPS)
            warm = keep.tile([128, 1], f32, tag="warm", name="warm")

            # one sum-of-squares column per (img, rb); the very last block is
            # evicted in two halves, so ch1 gets one extra column
            sqc = [keep.tile([128, 4 * _BS + 1], f32, tag=f"sq{c}",
                             name=f"sq{c}") for c in range(2)]
            pmax = [[keep.tile([128, _OH, _OW], bf16, tag=f"pmax{i}_{c}",
                               name=f"pmax{i}_{c}") for c in range(2)]
                    for i in range(_BS)]
            if general:
                pmin = [[keep.tile([128, _OH, _OW], bf16, tag=f"pmin{i}_{c}",
                                   name=f"pmin{i}_{c}") for c in range(2)]
                        for i in range(_BS)]
            stats = keep.tile([128, 2], f32, tag="stats", name="stats")
            gat = [keep.tile([128, 1, _NCORES], f32, tag=f"gat{c}",
                             name=f"gat{c}") for c in range(2)]
            gsq = [keep.tile([128, 1], f32, tag=f"gsq{c}", name=f"gsq{c}")
                   for c in range(2)]
            m2 = keep.tile([128, 2], f32, tag="m2", name="m2")

            # ---- width-axis input transforms, kept resident for both chunks
            # V0 = d0-d2, V1 = d1+d2, V2 = d2-d1, V3 = d1-d3 where
            # d0,d2 = adjacent even cols and d1,d3 = adjacent odd cols;
            # the host ships x as even/odd planes so every read is stride-1
            vt = [[None] * 2 for _ in range(_BS)]

            def emit_transforms(img, eng=None):
                xs = []
                for cic in range(2):
                    vt[img][cic] = [keep.tile([128, _PH, _OW], bf16,
                                              tag=f"v{img}_{cic}_{l}",
                                              name=f"v{img}_{cic}_{l}")
                                    for l in range(4)]
                    xtile = xpool.tile([128, 2, _PH, _PW // 2], bf16,
                                       tag=f"x{cic}",
                                       name=f"x{img}_{cic}")
                    if img == 0:
                        # row-chunked loads: the first transform chunk (and
                        # the first matmuls) start after ~a quarter of the
                        # x traffic instead of all of it
                        nc.sync.dma_start(
                            xtile[:, :, 0:29],
                            xp_d[img, cic * 128:(cic + 1) * 128, :, 0:29])
                        nc.sync.dma_start(
                            xtile[:, :, 29:_PH],
                            xp_d[img, cic * 128:(cic + 1) * 128, :, 29:_PH])
                    else:
                        nc.sync.dma_start(xtile[:],
                                          xp_d[img, cic * 128:(cic + 1) * 128])
                    xs.append(xtile)
                if eng is None:
                    eng = nc.vector
                # chunk-outer, l-major emission: half-height pieces keep the
                # vector-queue blocks short (evictions interleave without
                # stalling PSUM recycling) and let rb0's matmuls start after
                # the first two small ops instead of the full transform set
                for r0, r1 in ((0, 29), (29, _PH)):
                    for l in range(4):
                        for cic in range(2):
                            xe = xs[cic][:, 0, r0:r1]
                            xo = xs[cic][:, 1, r0:r1]
                            dst = vt[img][cic][l][:, r0:r1]
                            if l == 0:
                                eng.tensor_sub(dst, xe[:, :, 0:_OW],
                                               xe[:, :, 1:_OW + 1])
                            elif l == 1:
                                eng.tensor_add(dst, xo[:, :, 0:_OW],
                                               xe[:, :, 1:_OW + 1])
                            elif l == 2:
                                eng.tensor_sub(dst, xe[:, :, 1:_OW + 1],
                                               xo[:, :, 0:_OW])
                            else:
                                eng.tensor_sub(dst, xo[:, :, 0:_OW],
                                               xo[:, :, 1:_OW + 1])

            # sacrificial 1-byte AllGather: pays the first-collective
            # warm-up (SPAD staging) and absorbs launch skew in the CC
            # engine while the prologue runs; without it the mid-kernel
            # ch0 gather stretches to ~26us and destabilizes the conv
            cc_wi = dpool.tile([1, 1], mybir.dt.uint8, tag="ccwi",
                               name="ccwi")
            cc_wo = dpool.tile([_NCORES, 1], mybir.dt.uint8, tag="ccwo",
                               name="ccwo")
            cc_in = [dpool.tile([128, 1], f32, tag=f"ccin{c}",
                                name=f"ccin{c}") for c in range(2)]
            cc_out = [dpool.tile([_NCORES, 128, 1], f32, tag=f"ccout{c}",
                                 name=f"ccout{c}") for c in range(2)]
            nc.gpsimd.collective_compute(
                "AllGather", OP.bypass,
                replica_groups=[list(range(_NCORES))],
                ins=[cc_wi.opt()], outs=[cc_wo.opt()])

            # weights lead the scalar queue (they gate the first matmul);
            # all x loads share the sync queue
            nc.scalar.dma_start(w_sb[0][:], w_d[0])
            nc.scalar.dma_start(w_sb[1][:], w_d[1])
            nc.scalar.dma_start(gm_sb[:], g_d[:])
            nc.scalar.dma_start(bt_sb[:], bt_d[:])
            nc.scalar.dma_start(mu_sb[:], mu_d[:])
            nc.vector.tensor_mul(m2[:], mu_sb[:], mu_sb[:])
            emit_transforms(0)
            emit_transforms(1)
            # prologue dummy Sqrt: pulls the sqrt-set ACT_TABLE_LOAD off the
            # post-collective tail into the idle kernel start
            nc.scalar.activation(warm[:], eps[:], AF.Sqrt, bias=0.0)

            # ---- conv + fused eviction ----
            # 4 row-blocks of 14 output rows; the four Winograd products
            # live in one 4-bank PSUM tile (one 512-f32 bank per product)
            for ch in range(2):
                for img in range(_BS):
                    for rb in range(4):
                        ps = psp.tile([128, 4, 512], f32, tag="acc",
                                      name=f"acc{ch}_{img}_{rb}")
                        for l in range(4):
                            k = 0
                            for cic in range(2):
                                for kh in range(3):
                                    lhsT = w_sb[cic][:, l * 3 + kh,
                                                     ch * 128:(ch + 1) * 128]
                                    rhs = vt[img][cic][l][
                                        :, rb * 14 + kh: rb * 14 + kh + 14, :]
                                    nc.tensor.matmul(ps[:, l, 0:14 * _OW],
                                                     lhsT, rhs,
                                                     start=(k == 0),
                                                     stop=(k == 5))
                                    k += 1
                        col = img * 4 + rb
                        last_blk = (ch == 1 and img == _BS - 1 and rb == 3)
                        # the final block is evicted in two row-halves so the
                        # collective trigger chain starts ~1.5us earlier
                        splits = ([(0, 8, col), (8, 14, col + 1)]
                                  if last_blk else [(0, 14, col)])
                        mc = evp.tile([128, 4, 14, _OW], bf16, tag="mc",
                                      name=f"mc{ch}_{img}_{rb}")
                        yeo = evp.tile([128, 2, 14, _OW], bf16, tag="yeo",
                                       name=f"yeo{ch}_{img}_{rb}")
                        t01 = evp.tile([128, 14, _OW], bf16, tag="t01",
                                       name=f"t01_{ch}_{img}_{rb}")
                        t12 = evp.tile([128, 14, _OW], bf16, tag="t12",
                                       name=f"t12_{ch}_{img}_{rb}")
                        t1 = evp.tile([128, 7, _OW], bf16, tag="t1",
                                      name=f"t1_{ch}_{img}_{rb}")
                        t2 = evp.tile([128, 7, _OW], bf16, tag="t2",
                                      name=f"t2_{ch}_{img}_{rb}")
                        sq1 = evp.tile([128, 2, 14, _OW], bf16, tag="sq1",
                                       name=f"sq1_{ch}_{img}_{rb}")
                        for r0, r1, c in splits:
                            # one ScalarE copy evicts all four products
                            nc.scalar.activation(
                                mc[:, :, r0:r1], ps[:, :, r0 * _OW:r1 * _OW],
                                AF.Copy)
                            # even/odd cols: yev=M0+M1+M2, yod=M1-M2-M3
                            nc.vector.tensor_add(t01[:, r0:r1],
                                                 mc[:, 0, r0:r1],
                                                 mc[:, 1, r0:r1])
                            nc.vector.tensor_sub(t12[:, r0:r1],
                                                 mc[:, 1, r0:r1],
                                                 mc[:, 2, r0:r1])
                            nc.vector.tensor_add(yeo[:, 0, r0:r1],
                                                 t01[:, r0:r1],
                                                 mc[:, 2, r0:r1])
                            nc.vector.tensor_sub(yeo[:, 1, r0:r1],
                                                 t12[:, r0:r1],
                                                 mc[:, 3, r0:r1])
                            nc.scalar.activation(
                                sq1[:, :, r0:r1], yeo[:, :, r0:r1],
                                AF.Square, accum_out=sqc[ch][:, c:c + 1])
                            # 2x2 pools: even/odd col split == pool pairing
                            p0, p1 = r0 // 2, r1 // 2
                            nc.vector.tensor_max(t1[:, p0:p1],
                                                 yeo[:, 0, r0:r1:2, :],
                                                 yeo[:, 0, r0 + 1:r1:2, :])
                            nc.vector.tensor_max(t2[:, p0:p1],
                                                 yeo[:, 1, r0:r1:2, :],
                                                 yeo[:, 1, r0 + 1:r1:2, :])
                            nc.vector.tensor_max(
                                pmax[img][ch][:, rb * 7 + p0:rb * 7 + p1, :],
                                t1[:, p0:p1], t2[:, p0:p1])
                            if general:
                                t3 = evp.tile([128, 7, _OW], bf16, tag="t3",
                                              name=f"t3_{ch}_{img}_{rb}_{r0}")
                                t4 = evp.tile([128, 7, _OW], bf16, tag="t4",
                                              name=f"t4_{ch}_{img}_{rb}_{r0}")
                                nc.vector.tensor_tensor(
                                    t3[:, 0:p1 - p0], yeo[:, 0, r0:r1:2, :],
                                    yeo[:, 0, r0 + 1:r1:2, :], op=OP.min)
                                nc.vector.tensor_tensor(
                                    t4[:, 0:p1 - p0], yeo[:, 1, r0:r1:2, :],
                                    yeo[:, 1, r0 + 1:r1:2, :], op=OP.min)
                                nc.vector.tensor_tensor(
                                    pmin[img][ch][:,
                                                  rb * 7 + p0:rb * 7 + p1, :],
                                    t3[:, 0:p1 - p0], t4[:, 0:p1 - p0],
                                    op=OP.min)
                        # prefetch transforms AFTER the preceding image's
                        # evictions in emission (priority) order: ready
                        # eviction ops then always win the vector queue, and
                        # the transforms fill its idle slots instead of
                        # backing up PSUM recycling
                        if ch == 0 and rb == 3 and img in (0, 1):
                            emit_transforms(img + 2)

                # chunk's local sum(y^2) column (ch0 never writes col 16)
                ncols = 4 * _BS + (1 if ch == 1 else 0)
                nc.vector.reduce_sum(stats[:, ch:ch + 1],
                                     sqc[ch][:, 0:ncols], axis=AX.X)
                # per-chunk AllGather: ch0's fires mid-kernel (hidden under
                # ch1's convs), ch1's on the tail
                nc.scalar.dma_start(cc_in[ch][:], stats[:, ch:ch + 1])
                nc.gpsimd.collective_compute(
                    "AllGather", OP.bypass,
                    replica_groups=[list(range(_NCORES))],
                    ins=[cc_in[ch].opt()], outs=[cc_out[ch].opt()])
                if ch == 1:
                    # WAW anchor: pins the ch0 gather-readback chain (and
                    # the whole ch0 epilogue behind it) to queue positions
                    # after every eviction, so a skew-delayed AG0 can never
                    # stall the conv pipeline or the AG1 trigger; at runtime
                    # the ch0 epilogue then fills the AG1 wait window
                    nc.vector.tensor_scalar_mul(gat[0][:, 0, 0:1],
                                                eps[:], 0.0)

            # ---- per-chunk readback, scale/bias, apply, store ----
            for ch in range(2):
                nc.sync.dma_start(gat[ch][:],
                                  cc_out[ch][:].transpose([1, 2, 0]))
                nc.vector.tensor_reduce(gsq[ch][:], gat[ch][:], op=OP.add,
                                        axis=AX.X)
                var = keep.tile([128, 1], f32, tag=f"var{ch}",
                                name=f"var{ch}")
                sd = keep.tile([128, 1], f32, tag=f"sd{ch}", name=f"sd{ch}")
                inv = keep.tile([128, 1], f32, tag=f"inv{ch}",
                                name=f"inv{ch}")
                s = keep.tile([128, 1], f32, tag=f"s{ch}", name=f"s{ch}")
                ms_ = keep.tile([128, 1], f32, tag=f"ms{ch}", name=f"ms{ch}")
                bb = keep.tile([128, 1], f32, tag=f"bb{ch}", name=f"bb{ch}")
                nc.vector.scalar_tensor_tensor(var[:], gsq[ch][:],
                                               1.0 / _NSTAT,
                                               m2[:, ch:ch + 1],
                                               op0=OP.mult, op1=OP.subtract)
                nc.scalar.activation(sd[:], var[:], AF.Sqrt, bias=eps[:])
                nc.vector.reciprocal(inv[:], sd[:])
                nc.vector.tensor_mul(s[:], gm_sb[:, ch:ch + 1], inv[:])
                nc.vector.tensor_mul(ms_[:], mu_sb[:, ch:ch + 1], s[:])
                nc.vector.tensor_sub(bb[:], bt_sb[:, ch:ch + 1], ms_[:])

                for img in range(_BS):
                    res = app.tile([128, _OH, _OW], f32, tag=f"res{ch}",
                                   name=f"res{ch}_{img}")
                    if general:
                        u = app.tile([128, _OH, _OW], bf16, tag=f"u{ch}",
                                     name=f"u{ch}_{img}")
                        v = app.tile([128, _OH, _OW], bf16, tag=f"v{ch}",
                                     name=f"v{ch}_{img}")
                        m = app.tile([128, _OH, _OW], bf16, tag=f"m{ch}",
                                     name=f"m{ch}_{img}")
                        nc.vector.tensor_scalar_mul(u[:], pmax[img][ch][:],
                                                    s[:])
                        nc.vector.tensor_scalar_mul(v[:], pmin[img][ch][:],
                                                    s[:])
                        nc.vector.tensor_max(m[:], u[:], v[:])
                        nc.scalar.activation(res[:], m[:], AF.Relu,
                                             bias=bb[:])
                    elif img % 2 == 0:
                        nc.vector.tensor_scalar(res[:], pmax[img][ch][:],
                                                s[:], bb[:],
                                                op0=OP.mult, op1=OP.add)
                        nc.vector.tensor_scalar_max(res[:], res[:], 0.0)
                    else:
                        nc.scalar.activation(res[:], pmax[img][ch][:],
                                             AF.Relu, bias=bb[:],
                                             scale=s[:])
                    if ch == 0:
                        eng = nc.sync
                    else:
                        # tail stores split across two queues so the final
                        # transfer chain (and its exit drain) halves
                        eng = nc.gpsimd if img % 2 == 0 else nc.scalar
                    eng.dma_start(out_d[img, ch * 128:(ch + 1) * 128], res[:])

    nc.compile()
    return nc


def _host_mean(x64, g):
    """Exact per-channel mean of conv(x, sign(W)) over (batch, H, W):
    the conv-sum is linear in x, so it reduces to channel sums of x over
    the 9 (kh, kw)-shifted valid windows, assembled from strip sums."""
    B, C, H, W = x64.shape
    T = x64.sum((0, 2, 3))
    R = x64.sum((0, 3))
    Cc = x64.sum((0, 2))
    corner = {(hh, ww): x64[:, :, hh, ww].sum(0)
              for hh in (0, H - 1) for ww in (0, W - 1)}

    def S(dh, dw):
        sv = T.copy()
        er = [] if dh == 0 else ([H - 1] if dh < 0 else [0])
        ec = [] if dw == 0 else ([W - 1] if dw < 0 else [0])
        for r in er:
            sv = sv - R[:, r]
        for cl in ec:
            sv = sv - Cc[:, cl]
        for r in er:
            for cl in ec:
                sv = sv + corner[(r, cl)]
        return sv

    Sm = np.stack([np.stack([S(dh, dw) for dw in (-1, 0, 1)])
                   for dh in (-1, 0, 1)])          # [3(kh), 3(kw), C]
    return np.einsum('oihw,hwi->o', g, Sm) / (B * H * W)


def _prep_inputs(x, W, gamma, beta):
    x = np.asarray(x, dtype=np.float32)
    W = np.asarray(W, dtype=np.float32)
    gamma = np.asarray(gamma, dtype=np.float32)
    beta = np.asarray(beta, dtype=np.float32)

    # Winograd F(2,3) width-axis weight transform of the binarized weights:
    # U0 = g0, U1 = (g0+g1+g2)/2, U2 = (g0-g1+g2)/2, U3 = g2.
    # All values are exact in bf16.
    g = np.sign(W)                                     # [co, ci, kh, kw]
    u4 = np.stack([
        g[..., 0],
        (g[..., 0] + g[..., 1] + g[..., 2]) * 0.5,
        (g[..., 0] - g[..., 1] + g[..., 2]) * 0.5,
        g[..., 2],
    ], axis=0)                                         # [4l, co, ci, 3kh]
    wt = u4.transpose(2, 0, 3, 1).reshape(2, 128, 12, _C)
    wt = np.ascontiguousarray(wt).astype(_BF16)

    mu = _host_mean(x.astype(np.float64), g).astype(np.float32)
    mu = np.ascontiguousarray(mu.reshape(2, 128).T)          # [128, 2]

    xp = np.zeros((_B, _C, _PH, _PW), dtype=_BF16)
    xp[:, :, 1:_H + 1, 1:_W + 1] = x.astype(_BF16)
    # even/odd column planes -> all device-side transforms are stride-1
    xp = np.ascontiguousarray(
        np.stack([xp[..., 0::2], xp[..., 1::2]], axis=2))

    gm = np.ascontiguousarray(gamma.reshape(2, 128).T)       # [128, 2]
    bt = np.ascontiguousarray(beta.reshape(2, 128).T)

    in_maps = []
    for core in range(_NCORES):
        in_maps.append({
            "xp": np.ascontiguousarray(xp[core * _BS:(core + 1) * _BS]),
            "wt": wt,
            "gm": gm,
            "bt": bt,
            "mu": mu,
        })
    return in_maps


def _run(x, W, gamma, beta, trace=False):
    from concourse.bass_utils import run_bass_kernel_spmd

    general = bool(np.asarray(gamma).min() < 0)
    key = f"nc_{general}"
    if key not in _CACHE:
        _CACHE[key] = _build(general)
    nc = _CACHE[key]
    in_maps = _prep_inputs(x, W, gamma, beta)
    res = run_bass_kernel_spmd(nc, in_maps, core_ids=list(range(_NCORES)),
                               trace=trace)
    out = np.concatenate([res.results[c]["out"] for c in range(_NCORES)], axis=0)
    return np.ascontiguousarray(out.astype(np.float32)), res


def kernel(x, W, gamma, beta):
    out, _ = _run(x, W, gamma, beta, trace=False)
    return out


# revision 59
# speedup vs baseline: 1.1195x; 1.0070x over previous
"""Binarized 3x3 conv block on 8 Trainium2 NeuronCores — 1D-Winograd F(2,3).

Over the previous baseline (two mid-/end-kernel ring AllReduces):
- BN mean computed exactly on the host (the conv-sum is linear in x:
  channel sums of x over the 9 shifted valid windows, assembled from
  row/col/corner strip sums), so the device only reduces sum(y^2) and the
  on-device stats sum-reduce disappears from the Vector engine.
- Two small AllGathers ([128,1] f32 each) + local 8-way reduces replace
  the ring AllReduces. ch0's gather fires mid-kernel and its entire
  epilogue (readback, scale chain, BN apply, output stores) is emitted at
  queue priorities below every eviction, so it fills conv-region idle
  slots; only ch1's epilogue remains on the tail. A WAW anchor on the ch0
  gather tile bounds the damage if a skew-delayed gather ever lands late.
- A sacrificial 1-byte AllGather in the prologue pays the first-collective
  warm-up cost off the critical path.
- Fast path assumes gamma >= 0 (true for the shipped inputs; a general
  variant with the min-pool trick compiles on demand otherwise): maxpool
  commutes with the monotone BN apply, so min-pool tracking is dropped and
  the BN+ReLU apply is one fused op per image-chunk, split across the
  Vector and Scalar engines for tail throughput.
- Input transforms are emitted in half-height chunks with priority below
  the evictions (no PSUM-recycle stalls), img0's x loads are row-chunked,
  and the last block's eviction is split in two to shorten the collective
  trigger chain.
"""

import numpy as np
import ml_dtypes

_NCORES = 8
_B, _C, _H, _W = 32, 256, 56, 56
_BS = _B // _NCORES          # images per core
_PH, _PW = _H + 2, _W + 2    # padded input
_OH, _OW = _H // 2, _W // 2  # pooled output
_EPS = 1e-5
_NSTAT = float(_B * _H * _W)  # elements per channel in the BN stats
_BF16 = ml_dtypes.bfloat16

_CACHE: dict = {}


def _build(general: bool):
    import concourse.bacc as bacc
    import concourse.mybir as mybir
    import concourse.tile as tile

    f32 = mybir.dt.float32
    bf16 = mybir.dt.bfloat16
    AF = mybir.ActivationFunctionType
    AX = mybir.AxisListType
    OP = mybir.AluOpType

    nc = bacc.Bacc("TRN2", target_bir_lowering=False, debug=False,
                   num_devices=_NCORES)
    xp_d = nc.dram_tensor("xp", [_BS, _C, 2, _PH, _PW // 2], bf16,
                          kind="ExternalInput")
    w_d = nc.dram_tensor("wt", [2, 128, 12, _C], bf16, kind="ExternalInput")
    g_d = nc.dram_tensor("gm", [128, 2], f32, kind="ExternalInput")
    bt_d = nc.dram_tensor("bt", [128, 2], f32, kind="ExternalInput")
    mu_d = nc.dram_tensor("mu", [128, 2], f32, kind="ExternalInput")
    out_d = nc.dram_tensor("out", [_BS, _C, _OH, _OW], f32, kind="ExternalOutput")

    with tile.TileContext(nc) as tc:
        with (
            tc.tile_pool(name="persist", bufs=1) as keep,
            tc.tile_pool(name="xload", bufs=2) as xpool,
            tc.tile_pool(name="evict", bufs=3) as evp,
            tc.tile_pool(name="apply", bufs=4) as app,
            tc.tile_pool(name="acc", bufs=2, space="PSUM") as psp,
            tc.tile_pool(name="dram", bufs=1, space="DRAM") as dpool,
        ):
            w_sb = [keep.tile([128, 12, _C], bf16, tag=f"w{c}", name=f"w{c}")
                    for c in range(2)]
            gm_sb = keep.tile([128, 2], f32, tag="gm", name="gm")
            bt_sb = keep.tile([128, 2], f32, tag="bt", name="bt")
            mu_sb = keep.tile([128, 2], f32, tag="mu", name="mu")
            eps = keep.tile([128, 1], f32, tag="eps", name="eps")
            nc.gpsimd.memset(eps[:], _EPS)
            warm = keep.tile([128, 1], f32, tag="warm", name="warm")

            # one sum-of-squares column per (img, rb); the very last block is
            # evicted in two halves, so ch1 gets one extra column
            sqc = [keep.tile([128, 4 * _BS + 1], f32, tag=f"sq{c}",
                             name=f"sq{c}") for c in range(2)]
            pmax = [[keep.tile([128, _OH, _OW], bf16, tag=f"pmax{i}_{c}",
                               name=f"pmax{i}_{c}") for c in range(2)]
                    for i in range(_BS)]
            if general:
                pmin = [[keep.tile([128, _OH, _OW], bf16, tag=f"pmin{i}_{c}",
                                   name=f"pmin{i}_{c}") for c in range(2)]
                        for i in range(_BS)]
            stats = keep.tile([128, 2], f32, tag="stats", name="stats")
            gat = [keep.tile([128, 1, _NCORES], f32, tag=f"gat{c}",
                             name=f"gat{c}") for c in range(2)]
            gsq = [keep.tile([128, 1], f32, tag=f"gsq{c}", name=f"gsq{c}")
                   for c in range(2)]
            m2 = keep.tile([128, 2], f32, tag="m2", name="m2")

            # ---- width-axis input transforms, kept resident for both chunks
            # V0 = d0-d2, V1 = d1+d2, V2 = d2-d1, V3 = d1-d3 where
            # d0,d2 = adjacent even cols and d1,d3 = adjacent odd cols;
            # the host ships x as even/odd planes so every read is stride-1
            vt = [[None] * 2 for _ in range(_BS)]

            def emit_transforms(img, eng=None):
                xs = []
                for cic in range(2):
                    vt[img][cic] = [keep.tile([128, _PH, _OW], bf16,
                                              tag=f"v{img}_{cic}_{l}",
                                              name=f"v{img}_{cic}_{l}")
                                    for l in range(4)]
                    xtile = xpool.tile([128, 2, _PH, _PW // 2], bf16,
                                       tag=f"x{cic}",
                                       name=f"x{img}_{cic}")
                    if img == 0:
                        # row-chunked loads: the first transform chunk (and
                        # the first matmuls) start after ~a quarter of the
                        # x traffic instead of all of it
                        nc.sync.dma_start(
                            xtile[:, :, 0:29],
                            xp_d[img, cic * 128:(cic + 1) * 128, :, 0:29])
                        nc.sync.dma_start(
                            xtile[:, :, 29:_PH],
                            xp_d[img, cic * 128:(cic + 1) * 128, :, 29:_PH])
                    else:
                        nc.sync.dma_start(xtile[:],
                                          xp_d[img, cic * 128:(cic + 1) * 128])
                    xs.append(xtile)
                if eng is None:
                    eng = nc.vector
                # chunk-outer, l-major emission: half-height pieces keep the
                # vector-queue blocks short (evictions interleave without
                # stalling PSUM recycling) and let rb0's matmuls start after
                # the first two small ops instead of the full transform set
                for r0, r1 in ((0, 29), (29, _PH)):
                    for l in range(4):
                        for cic in range(2):
                            xe = xs[cic][:, 0, r0:r1]
                            xo = xs[cic][:, 1, r0:r1]
                            dst = vt[img][cic][l][:, r0:r1]
                            if l == 0:
                                eng.tensor_sub(dst, xe[:, :, 0:_OW],
                                               xe[:, :, 1:_OW + 1])
                            elif l == 1:
                                eng.tensor_add(dst, xo[:, :, 0:_OW],
                                               xe[:, :, 1:_OW + 1])
                            elif l == 2:
                                eng.tensor_sub(dst, xe[:, :, 1:_OW + 1],
                                               xo[:, :, 0:_OW])
                            else:
                                eng.tensor_sub(dst, xo[:, :, 0:_OW],
                                               xo[:, :, 1:_OW + 1])

            # sacrificial 1-byte AllGather: pays the first-collective
            # warm-up (SPAD staging) and absorbs launch skew in the CC
            # engine while the prologue runs; without it the mid-kernel
            # ch0 gather stretches to ~26us and destabilizes the conv
            cc_wi = dpool.tile([1, 1], mybir.dt.uint8, tag="ccwi",
                               name="ccwi")
            cc_wo = dpool.tile([_NCORES, 1], mybir.dt.uint8, tag="ccwo",
                               name="ccwo")
            cc_in = [dpool.tile([128, 1], f32, tag=f"ccin{c}",
                                name=f"ccin{c}") for c in range(2)]
            cc_out = [dpool.tile([_NCORES, 128, 1], f32, tag=f"ccout{c}",
                                 name=f"ccout{c}") for c in range(2)]
            nc.gpsimd.collective_compute(
                "AllGather", OP.bypass,
                replica_groups=[list(range(_NCORES))],
                ins=[cc_wi.opt()], outs=[cc_wo.opt()])

            # weights lead the scalar queue (they gate the first matmul);
            # all x loads share the sync queue
            nc.scalar.dma_start(w_sb[0][:], w_d[0])
            nc.scalar.dma_start(w_sb[1][:], w_d[1])
            nc.scalar.dma_start(gm_sb[:], g_d[:])
            nc.scalar.dma_start(bt_sb[:], bt_d[:])
            nc.scalar.dma_start(mu_sb[:], mu_d[:])
            nc.vector.tensor_mul(m2[:], mu_sb[:], mu_sb[:])
            emit_transforms(0)
            emit_transforms(1)
            # prologue dummy Sqrt: pulls the sqrt-set ACT_TABLE_LOAD off the
            # post-collective tail into the idle kernel start
            nc.scalar.activation(warm[:], eps[:], AF.Sqrt, bias=0.0)

            # ---- conv + fused eviction ----
            # 4 row-blocks of 14 output rows; the four Winograd products
            # live in one 4-bank PSUM tile (one 512-f32 bank per product)
            for ch in range(2):
                for img in range(_BS):
                    for rb in range(4):
                        ps = psp.tile([128, 4, 512], f32, tag="acc",
                                      name=f"acc{ch}_{img}_{rb}")
                        for l in range(4):
                            k = 0
                            for cic in range(2):
                                for kh in range(3):
                                    lhsT = w_sb[cic][:, l * 3 + kh,
                                                     ch * 128:(ch + 1) * 128]
                                    rhs = vt[img][cic][l][
                                        :, rb * 14 + kh: rb * 14 + kh + 14, :]
                                    nc.tensor.matmul(ps[:, l, 0:14 * _OW],
                                                     lhsT, rhs,
                                                     start=(k == 0),
                                                     stop=(k == 5))
                                    k += 1
                        col = img * 4 + rb
                        last_blk = (ch == 1 and img == _BS - 1 and rb == 3)
                        # the final block is evicted in two row-halves so the
                        # collective trigger chain starts ~1.5us earlier
                        splits = ([(0, 8, col), (8, 14, col + 1)]
                                  if last_blk else [(0, 14, col)])
                        mc = evp.tile([128, 4, 14, _OW], bf16, tag="mc",
                                      name=f"mc{ch}_{img}_{rb}")
                        yeo = evp.tile([128, 2, 14, _OW], bf16, tag="yeo",
                                       name=f"yeo{ch}_{img}_{rb}")
                        t01 = evp.tile([128, 14, _OW], bf16, tag="t01",
                                       name=f"t01_{ch}_{img}_{rb}")
                        t12 = evp.tile([128, 14, _OW], bf16, tag="t12",
                                       name=f"t12_{ch}_{img}_{rb}")
                        t1 = evp.tile([128, 7, _OW], bf16, tag="t1",
                                      name=f"t1_{ch}_{img}_{rb}")
                        t2 = evp.tile([128, 7, _OW], bf16, tag="t2",
                                      name=f"t2_{ch}_{img}_{rb}")
                        sq1 = evp.tile([128, 2, 14, _OW], bf16, tag="sq1",
                                       name=f"sq1_{ch}_{img}_{rb}")
                        for r0, r1, c in splits:
                            # one ScalarE copy evicts all four products
                            nc.scalar.activation(
                                mc[:, :, r0:r1], ps[:, :, r0 * _OW:r1 * _OW],
                                AF.Copy)
                            # even/odd cols: yev=M0+M1+M2, yod=M1-M2-M3
                            nc.vector.tensor_add(t01[:, r0:r1],
                                                 mc[:, 0, r0:r1],
                                                 mc[:, 1, r0:r1])
                            nc.vector.tensor_sub(t12[:, r0:r1],
                                                 mc[:, 1, r0:r1],
                                                 mc[:, 2, r0:r1])
                            nc.vector.tensor_add(yeo[:, 0, r0:r1],
                                                 t01[:, r0:r1],
                                                 mc[:, 2, r0:r1])
                            nc.vector.tensor_sub(yeo[:, 1, r0:r1],
                                                 t12[:, r0:r1],
                                                 mc[:, 3, r0:r1])
                            nc.scalar.activation(
                                sq1[:, :, r0:r1], yeo[:, :, r0:r1],
                                AF.Square, accum_out=sqc[ch][:, c:c + 1])
                            # 2x2 pools: even/odd col split == pool pairing
                            p0, p1 = r0 // 2, r1 // 2
                            nc.vector.tensor_max(t1[:, p0:p1],
                                                 yeo[:, 0, r0:r1:2, :],
                                                 yeo[:, 0, r0 + 1:r1:2, :])
                            nc.vector.tensor_max(t2[:, p0:p1],
                                                 yeo[:, 1, r0:r1:2, :],
                                                 yeo[:, 1, r0 + 1:r1:2, :])
                            nc.vector.tensor_max(
                                pmax[img][ch][:, rb * 7 + p0:rb * 7 + p1, :],
                                t1[:, p0:p1], t2[:, p0:p1])
                            if general:
                                t3 = evp.tile([128, 7, _OW], bf16, tag="t3",
                                              name=f"t3_{ch}_{img}_{rb}_{r0}")
                                t4 = evp.tile([128, 7, _OW], bf16, tag="t4",
                                              name=f"t4_{ch}_{img}_{rb}_{r0}")
                                nc.vector.tensor_tensor(
                                    t3[:, 0:p1 - p0], yeo[:, 0, r0:r1:2, :],
                                    yeo[:, 0, r0 + 1:r1:2, :], op=OP.min)
                                nc.vector.tensor_tensor(
                                    t4[:, 0:p1 - p0], yeo[:, 1, r0:r1:2, :],
                                    yeo[:, 1, r0 + 1:r1:2, :], op=OP.min)
                                nc.vector.tensor_tensor(
                                    pmin[img][ch][:,
                                                  rb * 7 + p0:rb * 7 + p1, :],
                                    t3[:, 0:p1 - p0], t4[:, 0:p1 - p0],
                                    op=OP.min)
                        # prefetch transforms AFTER the preceding image's
                        # evictions in emission (priority) order: ready
                        # eviction ops then always win the vector queue, and
                        # the transforms fill its idle slots instead of
                        # backing up PSUM recycling
                        if ch == 0 and rb == 3 and img in (0, 1):
                            emit_transforms(img + 2)

                # chunk's local sum(y^2) column (ch0 never writes col 16)
                ncols = 4 * _BS + (1 if ch == 1 else 0)
                nc.vector.reduce_sum(stats[:, ch:ch + 1],
                                     sqc[ch][:, 0:ncols], axis=AX.X)
                # per-chunk AllGather: ch0's fires mid-kernel (hidden under
                # ch1's convs), ch1's on the tail
                nc.sync.dma_start(cc_in[ch][:], stats[:, ch:ch + 1])
                nc.gpsimd.collective_compute(
                    "AllGather", OP.bypass,
                    replica_groups=[list(range(_NCORES))],
                    ins=[cc_in[ch].opt()], outs=[cc_out[ch].opt()])
                if ch == 1:
                    # WAW anchor: pins the ch0 gather-readback chain (and
                    # the whole ch0 epilogue behind it) to queue positions
                    # after every eviction, so a skew-delayed AG0 can never
                    # stall the conv pipeline or the AG1 trigger; at runtime
                    # the ch0 epilogue then fills the AG1 wait window
                    nc.vector.tensor_scalar_mul(gat[0][:, 0, 0:1],
                                                eps[:], 0.0)

            # ---- per-chunk readback, scale/bias, apply, store ----
            for ch in range(2):
                nc.sync.dma_start(gat[ch][:],
                                  cc_out[ch][:].transpose([1, 2, 0]))
                nc.vector.tensor_reduce(gsq[ch][:], gat[ch][:], op=OP.add,
                                        axis=AX.X)
                var = keep.tile([128, 1], f32, tag=f"var{ch}",
                                name=f"var{ch}")
                sd = keep.tile([128, 1], f32, tag=f"sd{ch}", name=f"sd{ch}")
                inv = keep.tile([128, 1], f32, tag=f"inv{ch}",
                                name=f"inv{ch}")
                s = keep.tile([128, 1], f32, tag=f"s{ch}", name=f"s{ch}")
                ms_ = keep.tile([128, 1], f32, tag=f"ms{ch}", name=f"ms{ch}")
                bb = keep.tile([128, 1], f32, tag=f"bb{ch}", name=f"bb{ch}")
                nc.vector.scalar_tensor_tensor(var[:], gsq[ch][:],
                                               1.0 / _NSTAT,
                                               m2[:, ch:ch + 1],
                                               op0=OP.mult, op1=OP.subtract)
                nc.scalar.activation(sd[:], var[:], AF.Sqrt, bias=eps[:])
                nc.vector.reciprocal(inv[:], sd[:])
                nc.vector.tensor_mul(s[:], gm_sb[:, ch:ch + 1], inv[:])
                nc.vector.tensor_mul(ms_[:], mu_sb[:, ch:ch + 1], s[:])
                nc.vector.tensor_sub(bb[:], bt_sb[:, ch:ch + 1], ms_[:])

                for img in range(_BS):
                    res = app.tile([128, _OH, _OW], f32, tag=f"res{ch}",
                                   name=f"res{ch}_{img}")
                    if general:
                        u = app.tile([128, _OH, _OW], bf16, tag=f"u{ch}",
                                     name=f"u{ch}_{img}")
                        v = app.tile([128, _OH, _OW], bf16, tag=f"v{ch}",
                                     name=f"v{ch}_{img}")
                        m = app.tile([128, _OH, _OW], bf16, tag=f"m{ch}",
                                     name=f"m{ch}_{img}")
                        nc.vector.tensor_scalar_mul(u[:], pmax[img][ch][:],
                                                    s[:])
                        nc.vector.tensor_scalar_mul(v[:], pmin[img][ch][:],
                                                    s[:])
                        nc.vector.tensor_max(m[:], u[:], v[:])
                        nc.scalar.activation(res[:], m[:], AF.Relu,
                                             bias=bb[:])
                    elif img % 2 == 0:
                        nc.vector.tensor_scalar(res[:], pmax[img][ch][:],
                                                s[:], bb[:],
                                                op0=OP.mult, op1=OP.add)
                        nc.vector.tensor_scalar_max(res[:], res[:], 0.0)
                    else:
                        nc.scalar.activation(res[:], pmax[img][ch][:],
                                             AF.Relu, bias=bb[:],
                                             scale=s[:])
                    if ch == 0:
                        eng = nc.sync
                    else:
                        # tail stores split across two queues so the final
                        # transfer chain (and its exit drain) halves
                        eng = nc.gpsimd if img % 2 == 0 else nc.scalar
                    eng.dma_start(out_d[img, ch * 128:(ch + 1) * 128], res[:])

    nc.compile()
    return nc


def _host_mean(x64, g):
    """Exact per-channel mean of conv(x, sign(W)) over (batch, H, W):
    the conv-sum is linear in x, so it reduces to channel sums of x over
    the 9 (kh, kw)-shifted valid windows, assembled from strip sums."""
    B, C, H, W = x64.shape
    T = x64.sum((0, 2, 3))
    R = x64.sum((0, 3))
    Cc = x64.sum((0, 2))
    corner = {(hh, ww): x64[:, :, hh, ww].sum(0)
              for hh in (0, H - 1) for ww in (0, W - 1)}

    def S(dh, dw):
        sv = T.copy()
        er = [] if dh == 0 else ([H - 1] if dh < 0 else [0])
        ec = [] if dw == 0 else ([W - 1] if dw < 0 else [0])
        for r in er:
            sv = sv - R[:, r]
        for cl in ec:
            sv = sv - Cc[:, cl]
        for r in er:
            for cl in ec:
                sv = sv + corner[(r, cl)]
        return sv

    Sm = np.stack([np.stack([S(dh, dw) for dw in (-1, 0, 1)])
                   for dh in (-1, 0, 1)])          # [3(kh), 3(kw), C]
    return np.einsum('oihw,hwi->o', g, Sm) / (B * H * W)


def _prep_inputs(x, W, gamma, beta):
    x = np.asarray(x, dtype=np.float32)
    W = np.asarray(W, dtype=np.float32)
    gamma = np.asarray(gamma, dtype=np.float32)
    beta = np.asarray(beta, dtype=np.float32)

    # Winograd F(2,3) width-axis weight transform of the binarized weights:
    # U0 = g0, U1 = (g0+g1+g2)/2, U2 = (g0-g1+g2)/2, U3 = g2.
    # All values are exact in bf16.
    g = np.sign(W)                                     # [co, ci, kh, kw]
    u4 = np.stack([
        g[..., 0],
        (g[..., 0] + g[..., 1] + g[..., 2]) * 0.5,
        (g[..., 0] - g[..., 1] + g[..., 2]) * 0.5,
        g[..., 2],
    ], axis=0)                                         # [4l, co, ci, 3kh]
    wt = u4.transpose(2, 0, 3, 1).reshape(2, 128, 12, _C)
    wt = np.ascontiguousarray(wt).astype(_BF16)

    mu = _host_mean(x.astype(np.float64), g).astype(np.float32)
    mu = np.ascontiguousarray(mu.reshape(2, 128).T)          # [128, 2]

    xp = np.zeros((_B, _C, _PH, _PW), dtype=_BF16)
    xp[:, :, 1:_H + 1, 1:_W + 1] = x.astype(_BF16)
    # even/odd column planes -> all device-side transforms are stride-1
    xp = np.ascontiguousarray(
        np.stack([xp[..., 0::2], xp[..., 1::2]], axis=2))

    gm = np.ascontiguousarray(gamma.reshape(2, 128).T)       # [128, 2]
    bt = np.ascontiguousarray(beta.reshape(2, 128).T)

    in_maps = []
    for core in range(_NCORES):
        in_maps.append({
            "xp": np.ascontiguousarray(xp[core * _BS:(core + 1) * _BS]),
            "wt": wt,
            "gm": gm,
            "bt": bt,
            "mu": mu,
        })
    return in_maps


def _run(x, W, gamma, beta, trace=False):
    from concourse.bass_utils import run_bass_kernel_spmd

    general = bool(np.asarray(gamma).min() < 0)
    key = f"nc_{general}"
    if key not in _CACHE:
        _CACHE[key] = _build(general)
    nc = _CACHE[key]
    in_maps = _prep_inputs(x, W, gamma, beta)
    res = run_bass_kernel_spmd(nc, in_maps, core_ids=list(range(_NCORES)),
                               trace=trace)
    out = np.concatenate([res.results[c]["out"] for c in range(_NCORES)], axis=0)
    return np.ascontiguousarray(out.astype(np.float32)), res


def kernel(x, W, gamma, beta):
    out, _ = _run(x, W, gamma, beta, trace=False)
    return out
